# revision 1
# baseline (speedup 1.0000x reference)
"""BinaryMLP (nn_BinaryMLP_91276644974884) on 8 TRN2 NeuronCores.

Reference network (B=32768, D=784, H1=H2=4096, C=10):
    h  = x @ W1.T + b1                    # fc1
    h  = BN1(prelu(h, a1)) (batch stats)
    h  = sign(h) @ sign(W2).T             # fc2, binary GEMM
    h  = BN2(prelu(h, a2))
    o  = log_softmax(h @ W3.T + b3)

Strategy: data-parallel over batch (4096 rows/core), computed in a
transposed [features, batch] layout so BatchNorm stats are free-axis
reductions.

- fc1 uses an fp16 hi/lo split with 2^11 scaling packed into one K=2432
  contraction ([xh;xh;xl] vs [wh*S;wl*S;wh]) -> fp32-class precision
  (needed because BN1's output feeds sign()) at 16-bit TensorEngine
  speed.  The fc1 bias folds in as an extra contraction row.
- fc2 (the 1.1 TFLOP binary GEMM) runs in fp8e4 with DoubleRow perf
  mode (K=256 per matmul): +-1 is exact in fp8 and PSUM accumulates in
  fp32, so the result is EXACT at ~2x bf16 rate.
- BatchNorm stats use 5 uneven feature groups per layer ([8,8,8,6,2]
  m-tiles) with pipelined AllReduces; the tiny LAST group minimizes the
  serial AllReduce+finalize tail at each phase boundary.  Reduce and
  finalize are emitted ~1 m-iteration apart so no engine FIFO ever
  waits on an in-flight collective.
- fc3+log_softmax is folded into phase 2: BN2 is algebraically folded
  into W3 (w3s = scale2*W3 computed on device per group; the bias2
  contribution becomes a [C] bias via a tiny PE accumulation), and the
  [C,512] logit partials accumulate per feature group in PSUM then DVE
  into an SBUF accumulator while fc2 still streams.  Only the last
  group's contributions + softmax remain after fc2's last matmul.
- Phase transitions overlap: w2 prefetch during fc1, s1 SBUF-residency
  loads begin the moment fc1's xt pool frees, fc2 PSUM chains consume
  k-tiles in arrival order so only the final (k=30,31) pair waits on
  the last BN1 group's sign pass.

Host-side prep (free - not on device critical path): transposes/blocked
weight layouts, sign(W2) cast to fp8, fp16 hi/lo splits.
"""

import contextlib

import numpy as np
import ml_dtypes

import concourse.bass as bass
import concourse.tile as tile
from concourse import bacc, mybir
from concourse.bass_utils import run_bass_kernel_spmd

F32 = mybir.dt.float32
F16 = mybir.dt.float16
F8 = mybir.dt.float8e4
AF = mybir.ActivationFunctionType
ALU = mybir.AluOpType

NCORES = 8
B = 32768
BS = B // NCORES          # 4096 batch rows per core
D = 784
K1ROWS = 2 * (D + 1) + D  # 2354: [xh+bias; xh+bias; xl] tightly packed along K
KC1 = -(-K1ROWS // 128)   # 19 chunks (padded to 2432)
FSPLIT = 2048.0           # 2^11 hi/lo split scale
H1 = 4096
H2 = 4096
MT = 32                   # 4096 / 128 feature tiles
C = 10
NB = BS // 512            # 8 512-col chunks per core
EPS = 1e-5
# BN stat groups (m-tile ranges). Tiny late groups -> short serial tail
# (the last groups' sign work otherwise spills past fc1's end and gates
# the phase transition).
GROUPS = [(0, 8), (8, 16), (16, 26), (26, 30), (30, 32)]
NGRP = len(GROUPS)
GMAX = max(m1 - m0 for m0, m1 in GROUPS)
QS = 1024                 # sign-pass batch-column chunk


def build_program(debug=False):
    nc = bacc.Bacc("TRN2", target_bir_lowering=False, debug=False,
                   num_devices=NCORES)

    xT = nc.declare_dram_parameter("xT", [128, NB, KC1, 512], F16,
                                   isOutput=False)
    w1 = nc.declare_dram_parameter("w1", [MT, 128, KC1, 128], F16, isOutput=False)
    w2 = nc.declare_dram_parameter("w2", [MT, 128, MT, 128], F8, isOutput=False)
    w3 = nc.declare_dram_parameter("w3", [128, MT, C], F16, isOutput=False)
    g1 = nc.declare_dram_parameter("g1", [128, MT], F32, isOutput=False)
    bt1 = nc.declare_dram_parameter("bt1", [128, MT], F32, isOutput=False)
    g2 = nc.declare_dram_parameter("g2", [128, MT], F32, isOutput=False)
    bt2 = nc.declare_dram_parameter("bt2", [128, MT], F32, isOutput=False)
    a1p = nc.declare_dram_parameter("a1p", [128, 1], F32, isOutput=False)
    a2p = nc.declare_dram_parameter("a2p", [128, 1], F32, isOutput=False)
    b3p = nc.declare_dram_parameter("b3p", [C, 1], F32, isOutput=False)
    c2n = nc.declare_dram_parameter("c2n", [128, MT], F32, isOutput=False)
    eye = nc.declare_dram_parameter("eye", [C, C], F32, isOutput=False)
    out = nc.declare_dram_parameter("out", [128, 4 * NB, C], F32, isOutput=True)

    with tile.TileContext(nc) as tc, contextlib.ExitStack() as es0:
        if True:
            const_pool = es0.enter_context(tc.tile_pool(name="const", bufs=1))
            stats_pool = es0.enter_context(tc.tile_pool(name="stats", bufs=1))
            dram_pool = es0.enter_context(
                tc.tile_pool(name="dram", bufs=1, space="DRAM"))
            ps_mm = es0.enter_context(
                tc.tile_pool(name="psmm", bufs=4, space="PSUM"))
            pin_pool = es0.enter_context(tc.tile_pool(name="pin", bufs=4))
            s1s_pool = es0.enter_context(tc.tile_pool(name="s1s", bufs=3))
            w2_pool = es0.enter_context(tc.tile_pool(name="w2p", bufs=2))
            # ---- persistent small tiles -------------------------------------
            g1_t = const_pool.tile([128, MT], F32, tag="g1")
            bt1_t = const_pool.tile([128, MT], F32, tag="bt1")
            g2_t = const_pool.tile([128, MT], F32, tag="g2")
            bt2_t = const_pool.tile([128, MT], F32, tag="bt2")
            a1_t = const_pool.tile([128, 1], F32, tag="a1")
            a2_t = const_pool.tile([128, 1], F32, tag="a2")
            b3_t = const_pool.tile([C, 1], F32, tag="b3")
            c2n_t = const_pool.tile([128, MT], F32, tag="c2n")
            eye_t = const_pool.tile([C, C], F32, tag="eye")
            w3_t = const_pool.tile([128, MT, C], F16, tag="w3")
            for t, d in [(g1_t, g1), (bt1_t, bt1), (g2_t, g2), (bt2_t, bt2),
                         (a1_t, a1p), (a2_t, a2p), (b3_t, b3p), (c2n_t, c2n),
                         (eye_t, eye), (w3_t, w3)]:
                nc.sync.dma_start(t[:], d.ap())

            sums1 = stats_pool.tile([128, MT, NB], F32, tag="sums1")
            sq1 = stats_pool.tile([128, MT, NB], F32, tag="sq1")
            sums2 = stats_pool.tile([128, MT, NB], F32, tag="sums2")
            sq2 = stats_pool.tile([128, MT, NB], F32, tag="sq2")

            p1d = dram_pool.tile([MT, 128, BS], F32, tag="p1d")
            p2d = dram_pool.tile([MT, 128, BS], F16, tag="p2d")
            s1d = dram_pool.tile([MT, 128, BS], F8, tag="s1d")
            cc_in1 = [dram_pool.tile([128, 2 * (m1 - m0)], F32,
                                     tag=f"cc_in1_{g}", name=f"cc_in1_{g}")
                      for g, (m0, m1) in enumerate(GROUPS)]
            cc_out1 = [dram_pool.tile([128, 2 * (m1 - m0)], F32,
                                      tag=f"cc_out1_{g}", name=f"cc_out1_{g}")
                       for g, (m0, m1) in enumerate(GROUPS)]
            cc_in2 = [dram_pool.tile([128, 2 * (m1 - m0)], F32,
                                     tag=f"cc_in2_{g}", name=f"cc_in2_{g}")
                      for g, (m0, m1) in enumerate(GROUPS)]
            cc_out2 = [dram_pool.tile([128, 2 * (m1 - m0)], F32,
                                      tag=f"cc_out2_{g}", name=f"cc_out2_{g}")
                       for g, (m0, m1) in enumerate(GROUPS)]

            scale1 = stats_pool.tile([128, MT], F32, tag="scale1")
            bias1 = stats_pool.tile([128, MT], F32, tag="bias1")
            negb1 = stats_pool.tile([128, MT], F32, tag="negb1")
            scale2 = stats_pool.tile([128, MT], F32, tag="scale2")
            bias2 = stats_pool.tile([128, MT], F32, tag="bias2")
            bias2h = stats_pool.tile([128, MT], F16, tag="bias2h")
            corr_t = stats_pool.tile([C, 1], F32, tag="corr")
            b3c_t = stats_pool.tile([C, 1], F32, tag="b3c")

            reds = {}

            def bn_reduce(sums, sq, cc_in, cc_out, g, tag):
                """Local group reduce + AllReduce launch (no finalize)."""
                m0, m1 = GROUPS[g]
                gl = m1 - m0
                msl = slice(m0, m1)
                cat = stats_pool.tile([128, 2 * gl], F32,
                                      tag=f"cat{tag}_{g}", name=f"cat{tag}_{g}")
                nc.vector.reduce_sum(cat[:, 0:gl], sums[:, msl, :],
                                     axis=mybir.AxisListType.X)
                nc.vector.reduce_sum(cat[:, gl:2 * gl], sq[:, msl, :],
                                     axis=mybir.AxisListType.X)
                nc.sync.dma_start(cc_in[g][:], cat[:])
                nc.gpsimd.collective_compute(
                    "AllReduce", ALU.add,
                    replica_groups=[list(range(NCORES))],
                    ins=[cc_in[g][:].opt()], outs=[cc_out[g][:].opt()],
                )
                red = stats_pool.tile([128, 2 * gl], F32,
                                      tag=f"red{tag}_{g}", name=f"red{tag}_{g}")
                nc.sync.dma_start(red[:], cc_out[g][:])
                reds[(tag, g)] = red

            def bn_finalize(g_t, bt_t, scale, bias, g, tag, negb=None):
                """Emitted >=1 m-iteration after bn_reduce so the DVE FIFO
                never waits on the in-flight collective."""
                m0, m1 = GROUPS[g]
                gl = m1 - m0
                msl = slice(m0, m1)
                red = reds[(tag, g)]
                mu = stats_pool.tile([128, GMAX], F32, tag=f"mu{tag}_{g}",
                                     name=f"mu{tag}_{g}")
                nc.vector.tensor_scalar_mul(mu[:, 0:gl], red[:, 0:gl], 1.0 / B)
                var = stats_pool.tile([128, GMAX], F32, tag=f"var{tag}_{g}",
                                      name=f"var{tag}_{g}")
                # var = E[p^2] - mu^2 + EPS
                nc.vector.tensor_mul(var[:, 0:gl], mu[:, 0:gl], mu[:, 0:gl])
                nc.vector.scalar_tensor_tensor(
                    var[:, 0:gl], red[:, gl:2 * gl], 1.0 / B, var[:, 0:gl],
                    ALU.mult, ALU.subtract,
                )
                nc.vector.tensor_scalar_add(var[:, 0:gl], var[:, 0:gl], EPS)
                rinv = stats_pool.tile([128, GMAX], F32, tag=f"rinv{tag}_{g}",
                                       name=f"rinv{tag}_{g}")
                nc.vector.reciprocal(rinv[:, 0:gl], var[:, 0:gl])
                r = stats_pool.tile([128, GMAX], F32, tag=f"r{tag}_{g}",
                                    name=f"r{tag}_{g}")
                nc.scalar.activation(r[:, 0:gl], rinv[:, 0:gl], AF.Sqrt)
                nc.vector.tensor_mul(scale[:, msl], g_t[:, msl], r[:, 0:gl])
                nc.vector.tensor_mul(bias[:, msl], mu[:, 0:gl], scale[:, msl])
                nc.vector.tensor_sub(bias[:, msl], bt_t[:, msl], bias[:, msl])
                if negb is not None:
                    nc.vector.tensor_scalar_mul(negb[:, msl], bias[:, msl],
                                                -1.0)

            # fc1-overlapped sign pass: p1d -> pin -> DVE (affine in-place,
            # is_ge) -> s1d, producing u = (scale1*p1+bias1 >= 0) in {1,0}.
            # The +-1 mapping is folded into fc2's Prelu (scale=2, bias=-corr
            # where corr = colsum(sign(W2)), known host-side).  Running on DVE
            # keeps the ScalarE FIFO free: the LAST group's tasks wait on the
            # final BN1 AllReduce, and on ScalarE that wait would head-of-line
            # block fc2's first prelu epilogues.
            # feature tiles k >= KDIR skip the s1d DRAM bounce: their signs
            # are computed straight into s1_t at phase-2 start.  k=28,29 use
            # +-1 encoding (ScalarE Sign; w2 host-halved to +-0.5 so the
            # fc2 epilogue's scale=2 stays uniform) since their BN stats are
            # ready early; k=30,31 use {0,1} on DVE (waits the final BN1
            # AllReduce without blocking the ScalarE FIFO).
            KDIR = 28
            sign_tasks = []

            def sign_group(g):
                for mm in range(GROUPS[g][0], min(GROUPS[g][1], KDIR)):
                    for q in range(BS // QS):
                        sign_tasks.append((mm, q))

            def emit_signs(k):
                # u = (p*scale >= -bias), one DVE op per chunk
                for _ in range(min(k, len(sign_tasks))):
                    mm, q = sign_tasks.pop(0)
                    pin = pin_pool.tile([128, QS], F32, tag="pin",
                                        name=f"pin_{mm}_{q}")
                    # pin triggers ride ScalarE; emitted at the TOP of each
                    # m-iteration with 4 pin bufs, their WAR waits reference
                    # the previous batch's (long-done) DVE reads, so they
                    # don't block the iteration's prelu/square ACTs.  On
                    # gpsimd they'd lockstep with the s1d writes (~4us/task).
                    nc.scalar.dma_start(
                        pin[:], p1d[mm, :, q * QS:(q + 1) * QS]
                    )
                    st = s1s_pool.tile([128, QS], F8, tag="s1s",
                                       name=f"s1s_{mm}_{q}")
                    nc.vector.tensor_scalar(
                        st[:], pin[:], scale1[:, mm:mm + 1],
                        negb1[:, mm:mm + 1], ALU.mult, ALU.is_ge,
                    )
                    nc.gpsimd.dma_start(
                        s1d[mm, :, q * QS:(q + 1) * QS], st[:]
                    )

            w2_tiles = {}

            def load_w2(m, eng):
                t = w2_pool.tile([128, MT, 128], F8, tag="w2", name=f"w2_{m}")
                for k0, k1 in ((0, 16), (16, MT)):
                    eng.dma_start(t[:, k0:k1, :], w2.ap()[m][:, k0:k1, :])
                w2_tiles[m] = t

            # ================= Phase 1: fc1 + prelu + stats ==================
            with contextlib.ExitStack() as es1:
                xt_pool = es1.enter_context(tc.tile_pool(name="xt", bufs=1))
                w1_pool = es1.enter_context(tc.tile_pool(name="w1p", bufs=2))
                p1_pool = es1.enter_context(tc.tile_pool(name="p1t", bufs=3))
                scr_pool = es1.enter_context(tc.tile_pool(name="scr1", bufs=2))
                # per-n tiles; first two n split finely so fc1 starts early
                xt_ts = []
                for n in range(NB):
                    xt_n = xt_pool.tile([128, KC1, 512], F16, tag=f"xt{n}",
                                        name=f"xt{n}")
                    if n < 2:
                        splits = [(k, k + 1) for k in range(KC1)]
                    else:
                        splits = [(0, 5), (5, 10), (10, 15), (15, KC1)]
                    for k0, k1 in splits:
                        nc.sync.dma_start(
                            xt_n[:, k0:k1, :], xT.ap()[:, n, k0:k1, :]
                        )
                    xt_ts.append(xt_n)
                for m in range(MT):
                    emit_signs(8)
                    w1_t = w1_pool.tile([128, KC1, 128], F16, tag="w1")
                    if m < 2:
                        eng = nc.gpsimd
                        splits = [(k, k + 1) for k in range(KC1)]
                    else:
                        eng = nc.sync
                        splits = [(0, 10), (10, KC1)]
                    for k0, k1 in splits:
                        eng.dma_start(
                            w1_t[:, k0:k1, :], w1.ap()[m][:, k0:k1, :]
                        )
                    for n in range(NB):
                        ps = ps_mm.tile([128, 512], F32, tag="mm")
                        for k in range(KC1):
                            nc.tensor.matmul(
                                ps[:], w1_t[:, k, :], xt_ts[n][:, k, :],
                                start=(k == 0), stop=(k == KC1 - 1),
                            )
                        p1_t = p1_pool.tile([128, 512], F32, tag="p1")
                        nc.scalar.activation(
                            p1_t[:], ps[:], AF.Prelu, alpha=a1_t[:],
                            scale=1.0 / FSPLIT,
                            accum_out=sums1[:, m, n:n + 1],
                        )
                        # p^2 sum on ScalarE (not DVE) so the vector FIFO stays
                        # free for BN finalize ops that wait on collectives
                        scr = scr_pool.tile([128, 512], F16, tag="scr")
                        nc.scalar.activation(
                            scr[:], p1_t[:], AF.Square,
                            accum_out=sq1[:, m, n:n + 1],
                        )
                        nc.sync.dma_start(
                            p1d[m, :, n * 512:(n + 1) * 512], p1_t[:]
                        )
                    for g in range(NGRP):
                        if m == GROUPS[g][1] - 1:
                            bn_reduce(sums1, sq1, cc_in1, cc_out1, g, "1")
                        if m == GROUPS[g][1] and g < NGRP - 1:
                            bn_finalize(g1_t, bt1_t, scale1, bias1, g, "1",
                                        negb1)
                            sign_group(g)
                    if m == 28:
                        load_w2(0, nc.sync)
                    if m == 29:
                        load_w2(1, nc.sync)
                emit_signs(8)
                # k >= KDIR tasks are handled in phase 2 (direct SBUF write)
                bn_finalize(g1_t, bt1_t, scale1, bias1, NGRP - 1, "1", negb1)

            # ============ Phase 2: fc2 + prelu + stats + fused fc3 ===========
            # m2-outer so W2 streams exactly once; s1 (fp8, 16.8 MB) is SBUF
            # resident (loads start the instant phase 1's xt pool frees).
            # fc3 partial chains interleave into the fc2 matmul stream.
            with contextlib.ExitStack() as es2:
                s1_pool = es2.enter_context(tc.tile_pool(name="s1", bufs=1))
                p2_pool = es2.enter_context(tc.tile_pool(name="p2t", bufs=4))
                scr2_pool = es2.enter_context(tc.tile_pool(name="sc2", bufs=3))
                q_pool = es2.enter_context(tc.tile_pool(name="qp", bufs=16))
                acc_pool = es2.enter_context(tc.tile_pool(name="acc", bufs=1))
                w3s_pool = es2.enter_context(tc.tile_pool(name="w3sp", bufs=1))
                ps3_pool = es2.enter_context(
                    tc.tile_pool(name="ps3", bufs=2, space="PSUM"))
                pcp_pool = es2.enter_context(
                    tc.tile_pool(name="pcp", bufs=1, space="PSUM"))
                pst_pool = es2.enter_context(
                    tc.tile_pool(name="pst", bufs=1, space="PSUM"))
                sm_pool = es2.enter_context(tc.tile_pool(name="sm", bufs=1))
                out_pool = es2.enter_context(tc.tile_pool(name="op", bufs=1))
                s1_t = s1_pool.tile([128, MT, BS], F8, tag="s1")
                for k in range(KDIR):
                    for h in range(2):
                        nc.sync.dma_start(
                            s1_t[:, k, h * 2048:(h + 1) * 2048],
                            s1d[k, :, h * 2048:(h + 1) * 2048],
                        )
                # k=28,29: +-1 via ScalarE Sign (stats ready; never waits)
                for mm in (28, 29):
                    for q in range(BS // QS):
                        pin = pin_pool.tile([128, QS], F32, tag="pin",
                                            name=f"pind_{mm}_{q}")
                        nc.gpsimd.dma_start(
                            pin[:], p1d[mm, :, q * QS:(q + 1) * QS]
                        )
                        nc.scalar.activation(
                            s1_t[:, mm, q * QS:(q + 1) * QS], pin[:], AF.Sign,
                            bias=bias1[:, mm:mm + 1],
                            scale=scale1[:, mm:mm + 1],
                        )
                # k=30,31: {0,1} via DVE is_ge (waits the last BN1 AllReduce
                # on the otherwise-idle vector FIFO)
                for mm in (30, 31):
                    for q in range(BS // QS):
                        pin = pin_pool.tile([128, QS], F32, tag="pin",
                                            name=f"pind_{mm}_{q}")
                        nc.gpsimd.dma_start(
                            pin[:], p1d[mm, :, q * QS:(q + 1) * QS]
                        )
                        nc.vector.tensor_scalar(
                            s1_t[:, mm, q * QS:(q + 1) * QS], pin[:],
                            scale1[:, mm:mm + 1], negb1[:, mm:mm + 1],
                            ALU.mult, ALU.is_ge,
                        )
                acc_t = acc_pool.tile([C, NB, 512], F32, tag="acc")
                w3s_t = w3s_pool.tile([128, MT, C], F16, tag="w3s")

                def bn2_extras(g):
                    """Per-group BN2-fold: w3s = scale2*W3, corr += W3^T bias2."""
                    m0, m1 = GROUPS[g]
                    nc.vector.tensor_copy(bias2h[:, m0:m1], bias2[:, m0:m1])
                    for k in range(m0, m1):
                        nc.vector.tensor_scalar_mul(
                            w3s_t[:, k, :], w3_t[:, k, :], scale2[:, k:k + 1]
                        )
                    pcp = pcp_pool.tile([C, 1], F32, tag="pcp")
                    for i, k in enumerate(range(m0, m1)):
                        nc.tensor.matmul(
                            pcp[:], w3_t[:, k, :], bias2h[:, k:k + 1],
                            start=(i == 0), stop=(k == m1 - 1),
                        )
                    if g == 0:
                        nc.vector.tensor_copy(corr_t[:], pcp[:])
                    else:
                        nc.vector.tensor_add(corr_t[:], corr_t[:], pcp[:])

                fc3_pend = []

                def emit_fc3_chain(g, n):
                    m0, m1 = GROUPS[g]
                    qts = []
                    for k in range(m0, m1):
                        qt = q_pool.tile([128, 512], F16, tag="q",
                                         name=f"q_{g}_{n}_{k}")
                        nc.gpsimd.dma_start(
                            qt[:], p2d[k, :, n * 512:(n + 1) * 512]
                        )
                        qts.append(qt)
                    pl = ps3_pool.tile([C, 512], F32, tag="pl")
                    for i, k in enumerate(range(m0, m1)):
                        nc.tensor.matmul(
                            pl[:], w3s_t[:, k, :], qts[i][:],
                            start=(i == 0), stop=(k == m1 - 1),
                        )
                    if g == 0:
                        nc.vector.tensor_copy(acc_t[:, n, :], pl[:])
                    else:
                        nc.vector.tensor_add(acc_t[:, n, :], acc_t[:, n, :],
                                             pl[:])

                for m in range(MT):
                    if m not in w2_tiles:
                        load_w2(m, nc.sync)
                    w2_t = w2_tiles.pop(m)
                    for n_g in range(NB):
                        ps = ps_mm.tile([128, 512], F32, tag="mm")
                        for kk in range(MT // 2):
                            nc.tensor.matmul(
                                ps[:], w2_t[:, 2 * kk:2 * kk + 2, :],
                                s1_t[:, 2 * kk:2 * kk + 2,
                                     n_g * 512:(n_g + 1) * 512],
                                start=(kk == 0), stop=(kk == MT // 2 - 1),
                                perf_mode=mybir.MatmulPerfMode.DoubleRow,
                            )
                        p2_t = p2_pool.tile([128, 512], F16, tag="p2")
                        # h2 = 2*(u @ sW2^T) - colsum(sW2): exact (even ints)
                        nc.scalar.activation(
                            p2_t[:], ps[:], AF.Prelu, alpha=a2_t[:],
                            scale=2.0, bias=c2n_t[:, m:m + 1],
                            accum_out=sums2[:, m, n_g:n_g + 1],
                        )
                        scr = scr2_pool.tile([128, 512], F16, tag="scr2")
                        nc.scalar.activation(
                            scr[:], p2_t[:], AF.Square,
                            accum_out=sq2[:, m, n_g:n_g + 1],
                        )
                        nc.sync.dma_start(
                            p2d[m, :, n_g * 512:(n_g + 1) * 512], p2_t[:]
                        )
                    for g in range(NGRP):
                        if m == GROUPS[g][1] - 1:
                            bn_reduce(sums2, sq2, cc_in2, cc_out2, g, "2")
                        if m == GROUPS[g][1] and g < NGRP - 1:
                            bn_finalize(g2_t, bt2_t, scale2, bias2, g, "2")
                            bn2_extras(g)
                        if m == GROUPS[g][1] + 1 and g < NGRP - 1:
                            fc3_pend.extend((g, n) for n in range(NB))
                    cap = len(fc3_pend) if m == MT - 1 else 3
                    for _ in range(min(cap, len(fc3_pend))):
                        emit_fc3_chain(*fc3_pend.pop(0))

                # ---------- tail: last-group fc3 + log_softmax ----------------
                bn_finalize(g2_t, bt2_t, scale2, bias2, NGRP - 1, "2")
                bn2_extras(NGRP - 1)
                fc3_pend.extend((NGRP - 1, n) for n in range(NB))
                for g, n in fc3_pend:
                    emit_fc3_chain(g, n)
                fc3_pend = []
                # ---- bulk log_softmax on one PE-transposed [128, 32, C]
                # block.  Logits are O(+-8) so exp() needs no max-shift in
                # fp32; the per-row lse subtraction rides the ACT bias port.
                nc.vector.tensor_add(b3c_t[:], b3_t[:], corr_t[:])
                nc.vector.tensor_scalar(
                    acc_t[:], acc_t[:], b3c_t[:], None, ALU.add
                )
                JJ = 4 * NB
                ptall = pst_pool.tile([128, JJ, C], F32, tag="pt")
                for n in range(NB):
                    for j in range(4):
                        nc.tensor.transpose(
                            ptall[:, n * 4 + j, :],
                            acc_t[:, n, j * 128:(j + 1) * 128], eye_t[:]
                        )
                ex2 = sm_pool.tile([128, JJ, C], F32, tag="ex2")
                nc.scalar.activation(ex2[:], ptall[:], AF.Exp)
                sen = sm_pool.tile([128, JJ], F32, tag="se")
                nc.vector.reduce_sum(sen[:], ex2[:], axis=mybir.AxisListType.X)
                lnn = sm_pool.tile([128, JJ], F32, tag="ln")
                nc.scalar.activation(lnn[:], sen[:], AF.Ln)
                nln = sm_pool.tile([128, JJ], F32, tag="nln")
                nc.vector.tensor_scalar_mul(nln[:], lnn[:], -1.0)
                ot = out_pool.tile([128, JJ, C], F32, tag="ot")
                for jj in range(JJ):
                    nc.scalar.activation(
                        ot[:, jj, :], ptall[:, jj, :], AF.Identity,
                        bias=nln[:, jj:jj + 1],
                    )
                nc.sync.dma_start(out.ap(), ot[:])

    nc.compile()
    return nc


def prep_inputs(x, W1, b1, a1, g1, beta1, W2, a2, g2, beta2, W3, b3):
    """Host-side layout prep. Returns per-core in_maps."""
    x = np.ascontiguousarray(np.asarray(x, np.float32))
    W1 = np.asarray(W1, np.float32)
    b1 = np.asarray(b1, np.float32)
    W2 = np.asarray(W2, np.float32)
    W3 = np.asarray(W3, np.float32)
    b3 = np.asarray(b3, np.float32)

    # fc1 operands with bias folded in as contraction row 784 (rows 785+ zero).
    # fp16 hi/lo split with 2^11 scaling, packed along K:
    #   XF = [xh; xh; xl*S],  WF = [wh*S; wl*S; wh]  ->  psum = S * h1
    # where v = vh + vl exactly captures ~22 mantissa bits.  The bias row uses
    # x-side 32.0 / w-side b1/32 to keep w*S within fp16 range.
    S = np.float32(FSPLIT)
    xT_aug = np.zeros((D + 1, B), np.float32)
    xT_aug[0:D] = x.T
    xT_aug[D] = 32.0
    w1T_aug = np.zeros((D + 1, H1), np.float32)
    w1T_aug[0:D] = W1.T
    w1T_aug[D] = b1 / 32.0

    xh = xT_aug.astype(np.float16)
    xl = ((xT_aug - xh.astype(np.float32)) * S).astype(np.float16)
    wh = w1T_aug.astype(np.float16)
    whs = (w1T_aug * S).astype(np.float16)
    wls = ((w1T_aug - wh.astype(np.float32)) * S).astype(np.float16)
    KPAD = KC1 * 128
    A = D + 1
    xF = np.zeros((KPAD, B), np.float16)
    xF[0:A] = xh
    xF[A:2 * A] = xh
    xF[2 * A:2 * A + D] = xl[0:D]
    wF = np.zeros((KPAD, H1), np.float16)
    wF[0:A] = whs
    wF[A:2 * A] = wls
    wF[2 * A:2 * A + D] = wh[0:D]
    w1_blk = np.ascontiguousarray(
        wF.reshape(KC1, 128, MT, 128).transpose(2, 1, 0, 3)
    )

    # k < 28 and k in {30,31}: s1 encoded {0,1}, weights +-1, corrected via
    # c2n = -colsum.  k in {28,29}: s1 encoded +-1 with weights halved to
    # +-0.5 (the fc2 epilogue applies a uniform scale of 2).
    sW2T = np.where(W2 >= 0, np.float32(1), np.float32(-1)).T
    sW2Ts = sW2T.copy()
    sW2Ts[28 * 128:30 * 128] *= np.float32(0.5)
    w2_blk = np.ascontiguousarray(
        sW2Ts.reshape(MT, 128, MT, 128).transpose(2, 1, 0, 3)
    ).astype(ml_dtypes.float8_e4m3)
    c2n_blk = -(
        sW2T[0:28 * 128].sum(axis=0, dtype=np.float64)
        + sW2T[30 * 128:].sum(axis=0, dtype=np.float64)
    ).astype(np.float32)

    w3_blk = np.ascontiguousarray(
        W3.T.reshape(MT, 128, C).transpose(1, 0, 2)
    ).astype(np.float16)

    def feat_layout(v):
        return np.ascontiguousarray(np.asarray(v, np.float32).reshape(MT, 128).T)

    shared = dict(
        w1=w1_blk, w2=w2_blk, w3=w3_blk,
        g1=feat_layout(g1), bt1=feat_layout(beta1),
        g2=feat_layout(g2), bt2=feat_layout(beta2),
        a1p=np.full((128, 1), np.float32(a1), np.float32),
        a2p=np.full((128, 1), np.float32(a2), np.float32),
        b3p=b3.reshape(C, 1).astype(np.float32),
        c2n=feat_layout(c2n_blk),
        eye=np.eye(C, dtype=np.float32),
    )
    in_maps = []
    for c in range(NCORES):
        sl = xF[:, c * BS:(c + 1) * BS]
        xs = np.ascontiguousarray(
            sl.reshape(KC1, 128, NB, 512).transpose(1, 2, 0, 3)
        )
        in_maps.append(dict(shared, xT=xs))
    return in_maps


_NC_CACHE = {}


def run(inputs, debug=False, trace=False):
    key = (debug,)
    if key not in _NC_CACHE:
        _NC_CACHE[key] = build_program(debug=debug)
    nc = _NC_CACHE[key]
    in_maps = prep_inputs(**inputs)
    res = run_bass_kernel_spmd(
        nc, in_maps, core_ids=list(range(NCORES)), trace=trace
    )
    # out is [128, 32, C] partition-major; row jj*128+p <-> out[p, jj]
    outs = np.concatenate([
        np.transpose(res.results[c]["out"], (1, 0, 2)).reshape(BS, C)
        for c in range(NCORES)
    ], axis=0)
    return outs, res


def kernel(**inputs):
    out, _ = run(inputs)
    return out



# revision 15
# speedup vs baseline: 1.0439x; 1.0439x over previous
"""BinaryMLP (nn_BinaryMLP_91276644974884) on 8 TRN2 NeuronCores.

Reference network (B=32768, D=784, H1=H2=4096, C=10):
    h  = x @ W1.T + b1                    # fc1
    h  = BN1(prelu(h, a1)) (batch stats)
    h  = sign(h) @ sign(W2).T             # fc2, binary GEMM
    h  = BN2(prelu(h, a2))
    o  = log_softmax(h @ W3.T + b3)

Strategy: data-parallel over batch (4096 rows/core), computed in a
transposed [features, batch] layout so BatchNorm stats are free-axis
reductions.

- fc1 uses an fp16 hi/lo split with 2^11 scaling packed into one K=2432
  contraction ([xh;xh;xl] vs [wh*S;wl*S;wh]) -> fp32-class precision
  (needed because BN1's output feeds sign()) at 16-bit TensorEngine
  speed.  The fc1 bias folds in as an extra contraction row.
- fc2 (the 1.1 TFLOP binary GEMM) runs in fp8e4 with DoubleRow perf
  mode (K=256 per matmul): +-1 is exact in fp8 and PSUM accumulates in
  fp32, so the result is EXACT at ~2x bf16 rate.
- BatchNorm stats use 5 uneven feature groups per layer ([8,8,8,6,2]
  m-tiles) with pipelined AllReduces; the tiny LAST group minimizes the
  serial AllReduce+finalize tail at each phase boundary.  Reduce and
  finalize are emitted ~1 m-iteration apart so no engine FIFO ever
  waits on an in-flight collective.
- fc3+log_softmax is folded into phase 2: BN2 is algebraically folded
  into W3 (w3s = scale2*W3 computed on device per group; the bias2
  contribution becomes a [C] bias via a tiny PE accumulation), and the
  [C,512] logit partials accumulate per feature group in PSUM then DVE
  into an SBUF accumulator while fc2 still streams.  Only the last
  group's contributions + softmax remain after fc2's last matmul.
- Phase transitions overlap: w2 prefetch during fc1, s1 SBUF-residency
  loads begin the moment fc1's xt pool frees, fc2 PSUM chains consume
  k-tiles in arrival order so only the final (k=30,31) pair waits on
  the last BN1 group's sign pass.

Host-side prep (free - not on device critical path): transposes/blocked
weight layouts, sign(W2) cast to fp8, fp16 hi/lo splits.
"""

import contextlib

import numpy as np
import ml_dtypes

import concourse.bass as bass
import concourse.tile as tile
from concourse import bacc, mybir
from concourse.bass_utils import run_bass_kernel_spmd

F32 = mybir.dt.float32
F16 = mybir.dt.float16
F8 = mybir.dt.float8e4
AF = mybir.ActivationFunctionType
ALU = mybir.AluOpType

NCORES = 8
B = 32768
BS = B // NCORES          # 4096 batch rows per core
D = 784
K1ROWS = 2 * (D + 1)      # 1570: [xh+bias; xl*S] 2-term split packed along K
KC1 = -(-K1ROWS // 128)   # 13 chunks (padded to 1664)
NCORR = 4                 # fp8 DoubleRow correction matmuls (covers all 785 rows)
FSPLIT = 262144.0         # 2^18 split scale (shared by fp16 + fp8 terms)
H1 = 4096
H2 = 4096
MT = 32                   # 4096 / 128 feature tiles
C = 10
NB = BS // 512            # 8 512-col chunks per core
EPS = 1e-5
# BN stat groups (m-tile ranges). Tiny late groups -> short serial tail
# (the last groups' sign work otherwise spills past fc1's end and gates
# the phase transition).
GROUPS = [(0, 8), (8, 16), (16, 26), (26, 30), (30, 32)]
NGRP = len(GROUPS)
GMAX = max(m1 - m0 for m0, m1 in GROUPS)
QS = 1024                 # sign-pass batch-column chunk


def build_program(debug=False):
    nc = bacc.Bacc("TRN2", target_bir_lowering=False, debug=False,
                   num_devices=NCORES)

    xT = nc.declare_dram_parameter("xT", [128, NB, KC1, 512], F16,
                                   isOutput=False)
    x8T = nc.declare_dram_parameter("x8T", [128, NB, 2 * NCORR, 512], F8,
                                    isOutput=False)
    w1 = nc.declare_dram_parameter("w1", [MT, 128, KC1, 128], F16, isOutput=False)
    w1c = nc.declare_dram_parameter("w1c", [MT, 128, 2 * NCORR, 128], F8,
                                    isOutput=False)
    w2 = nc.declare_dram_parameter("w2", [MT, 128, MT, 128], F8, isOutput=False)
    w3 = nc.declare_dram_parameter("w3", [128, MT, C], F16, isOutput=False)
    g1 = nc.declare_dram_parameter("g1", [128, MT], F32, isOutput=False)
    bt1 = nc.declare_dram_parameter("bt1", [128, MT], F32, isOutput=False)
    g2 = nc.declare_dram_parameter("g2", [128, MT], F32, isOutput=False)
    bt2 = nc.declare_dram_parameter("bt2", [128, MT], F32, isOutput=False)
    a1p = nc.declare_dram_parameter("a1p", [128, 1], F32, isOutput=False)
    a2p = nc.declare_dram_parameter("a2p", [128, 1], F32, isOutput=False)
    b3p = nc.declare_dram_parameter("b3p", [C, 1], F32, isOutput=False)
    c2n = nc.declare_dram_parameter("c2n", [128, MT], F32, isOutput=False)
    eye = nc.declare_dram_parameter("eye", [C, C], F32, isOutput=False)
    out = nc.declare_dram_parameter("out", [128, 4 * NB, C], F32, isOutput=True)

    with tile.TileContext(nc) as tc, contextlib.ExitStack() as es0:
        if True:
            const_pool = es0.enter_context(tc.tile_pool(name="const", bufs=1))
            stats_pool = es0.enter_context(tc.tile_pool(name="stats", bufs=1))
            dram_pool = es0.enter_context(
                tc.tile_pool(name="dram", bufs=1, space="DRAM"))
            ps_mm = es0.enter_context(
                tc.tile_pool(name="psmm", bufs=4, space="PSUM"))
            pin_pool = es0.enter_context(tc.tile_pool(name="pin", bufs=4))
            s1s_pool = es0.enter_context(tc.tile_pool(name="s1s", bufs=3))
            w2_pool = es0.enter_context(tc.tile_pool(name="w2p", bufs=2))
            # ---- persistent small tiles -------------------------------------
            g1_t = const_pool.tile([128, MT], F32, tag="g1")
            bt1_t = const_pool.tile([128, MT], F32, tag="bt1")
            g2_t = const_pool.tile([128, MT], F32, tag="g2")
            bt2_t = const_pool.tile([128, MT], F32, tag="bt2")
            a1_t = const_pool.tile([128, 1], F32, tag="a1")
            a2_t = const_pool.tile([128, 1], F32, tag="a2")
            b3_t = const_pool.tile([C, 1], F32, tag="b3")
            c2n_t = const_pool.tile([128, MT], F32, tag="c2n")
            eye_t = const_pool.tile([C, C], F32, tag="eye")
            w3_t = const_pool.tile([128, MT, C], F16, tag="w3")
            for t, d in [(g1_t, g1), (bt1_t, bt1), (g2_t, g2), (bt2_t, bt2),
                         (a1_t, a1p), (a2_t, a2p), (b3_t, b3p), (c2n_t, c2n),
                         (eye_t, eye), (w3_t, w3)]:
                nc.sync.dma_start(t[:], d.ap())

            sums1 = stats_pool.tile([128, MT, NB], F32, tag="sums1")
            sq1 = stats_pool.tile([128, MT, NB], F32, tag="sq1")
            sums2 = stats_pool.tile([128, MT, NB], F32, tag="sums2")
            sq2 = stats_pool.tile([128, MT, NB], F32, tag="sq2")

            p1d = dram_pool.tile([MT, 128, BS], F32, tag="p1d")
            p2d = dram_pool.tile([MT, 128, BS], F16, tag="p2d")
            s1d = dram_pool.tile([MT, 128, BS], F8, tag="s1d")
            cc_in1 = [dram_pool.tile([128, 2 * (m1 - m0)], F32,
                                     tag=f"cc_in1_{g}", name=f"cc_in1_{g}")
                      for g, (m0, m1) in enumerate(GROUPS)]
            cc_out1 = [dram_pool.tile([128, 2 * (m1 - m0)], F32,
                                      tag=f"cc_out1_{g}", name=f"cc_out1_{g}")
                       for g, (m0, m1) in enumerate(GROUPS)]
            cc_in2 = [dram_pool.tile([128, 2 * (m1 - m0)], F32,
                                     tag=f"cc_in2_{g}", name=f"cc_in2_{g}")
                      for g, (m0, m1) in enumerate(GROUPS)]
            cc_out2 = [dram_pool.tile([128, 2 * (m1 - m0)], F32,
                                      tag=f"cc_out2_{g}", name=f"cc_out2_{g}")
                       for g, (m0, m1) in enumerate(GROUPS)]

            scale1 = stats_pool.tile([128, MT], F32, tag="scale1")
            bias1 = stats_pool.tile([128, MT], F32, tag="bias1")
            negb1 = stats_pool.tile([128, MT], F32, tag="negb1")
            scale2 = stats_pool.tile([128, MT], F32, tag="scale2")
            bias2 = stats_pool.tile([128, MT], F32, tag="bias2")
            bias2h = stats_pool.tile([128, MT], F16, tag="bias2h")
            corr_t = stats_pool.tile([C, 1], F32, tag="corr")
            b3c_t = stats_pool.tile([C, 1], F32, tag="b3c")

            reds = {}

            def bn_reduce(sums, sq, cc_in, cc_out, g, tag):
                """Local group reduce + AllReduce launch (no finalize)."""
                m0, m1 = GROUPS[g]
                gl = m1 - m0
                msl = slice(m0, m1)
                cat = stats_pool.tile([128, 2 * gl], F32,
                                      tag=f"cat{tag}_{g}", name=f"cat{tag}_{g}")
                nc.vector.reduce_sum(cat[:, 0:gl], sums[:, msl, :],
                                     axis=mybir.AxisListType.X)
                nc.vector.reduce_sum(cat[:, gl:2 * gl], sq[:, msl, :],
                                     axis=mybir.AxisListType.X)
                nc.sync.dma_start(cc_in[g][:], cat[:])
                nc.gpsimd.collective_compute(
                    "AllReduce", ALU.add,
                    replica_groups=[list(range(NCORES))],
                    ins=[cc_in[g][:].opt()], outs=[cc_out[g][:].opt()],
                )
                red = stats_pool.tile([128, 2 * gl], F32,
                                      tag=f"red{tag}_{g}", name=f"red{tag}_{g}")
                nc.sync.dma_start(red[:], cc_out[g][:])
                reds[(tag, g)] = red

            def bn_finalize(g_t, bt_t, scale, bias, g, tag, negb=None):
                """Emitted >=1 m-iteration after bn_reduce so the DVE FIFO
                never waits on the in-flight collective."""
                m0, m1 = GROUPS[g]
                gl = m1 - m0
                msl = slice(m0, m1)
                red = reds[(tag, g)]
                mu = stats_pool.tile([128, GMAX], F32, tag=f"mu{tag}_{g}",
                                     name=f"mu{tag}_{g}")
                nc.vector.tensor_scalar_mul(mu[:, 0:gl], red[:, 0:gl], 1.0 / B)
                var = stats_pool.tile([128, GMAX], F32, tag=f"var{tag}_{g}",
                                      name=f"var{tag}_{g}")
                # var = E[p^2] - mu^2 + EPS
                nc.vector.tensor_mul(var[:, 0:gl], mu[:, 0:gl], mu[:, 0:gl])
                nc.vector.scalar_tensor_tensor(
                    var[:, 0:gl], red[:, gl:2 * gl], 1.0 / B, var[:, 0:gl],
                    ALU.mult, ALU.subtract,
                )
                nc.vector.tensor_scalar_add(var[:, 0:gl], var[:, 0:gl], EPS)
                rinv = stats_pool.tile([128, GMAX], F32, tag=f"rinv{tag}_{g}",
                                       name=f"rinv{tag}_{g}")
                nc.vector.reciprocal(rinv[:, 0:gl], var[:, 0:gl])
                r = stats_pool.tile([128, GMAX], F32, tag=f"r{tag}_{g}",
                                    name=f"r{tag}_{g}")
                nc.scalar.activation(r[:, 0:gl], rinv[:, 0:gl], AF.Sqrt)
                nc.vector.tensor_mul(scale[:, msl], g_t[:, msl], r[:, 0:gl])
                nc.vector.tensor_mul(bias[:, msl], mu[:, 0:gl], scale[:, msl])
                nc.vector.tensor_sub(bias[:, msl], bt_t[:, msl], bias[:, msl])
                if negb is not None:
                    nc.vector.tensor_scalar_mul(negb[:, msl], bias[:, msl],
                                                -1.0)

            # fc1-overlapped sign pass: p1d -> pin -> DVE (affine in-place,
            # is_ge) -> s1d, producing u = (scale1*p1+bias1 >= 0) in {1,0}.
            # The +-1 mapping is folded into fc2's Prelu (scale=2, bias=-corr
            # where corr = colsum(sign(W2)), known host-side).  Running on DVE
            # keeps the ScalarE FIFO free: the LAST group's tasks wait on the
            # final BN1 AllReduce, and on ScalarE that wait would head-of-line
            # block fc2's first prelu epilogues.
            # feature tiles k >= KDIR skip the s1d DRAM bounce: their signs
            # are computed straight into s1_t at phase-2 start.  k=28,29 use
            # +-1 encoding (ScalarE Sign; w2 host-halved to +-0.5 so the
            # fc2 epilogue's scale=2 stays uniform) since their BN stats are
            # ready early; k=30,31 use {0,1} on DVE (waits the final BN1
            # AllReduce without blocking the ScalarE FIFO).
            KDIR = 28
            sign_tasks = []

            def sign_group(g):
                for mm in range(GROUPS[g][0], min(GROUPS[g][1], KDIR)):
                    for q in range(BS // QS):
                        sign_tasks.append((mm, q))

            def emit_signs(k):
                # u = (p*scale >= -bias), one DVE op per chunk
                for _ in range(min(k, len(sign_tasks))):
                    mm, q = sign_tasks.pop(0)
                    pin = pin_pool.tile([128, QS], F32, tag="pin",
                                        name=f"pin_{mm}_{q}")
                    # pin triggers ride ScalarE; emitted at the TOP of each
                    # m-iteration with 4 pin bufs, their WAR waits reference
                    # the previous batch's (long-done) DVE reads, so they
                    # don't block the iteration's prelu/square ACTs.  On
                    # gpsimd they'd lockstep with the s1d writes (~4us/task).
                    nc.scalar.dma_start(
                        pin[:], p1d[mm, :, q * QS:(q + 1) * QS]
                    )
                    st = s1s_pool.tile([128, QS], F8, tag="s1s",
                                       name=f"s1s_{mm}_{q}")
                    nc.vector.tensor_scalar(
                        st[:], pin[:], scale1[:, mm:mm + 1],
                        negb1[:, mm:mm + 1], ALU.mult, ALU.is_ge,
                    )
                    nc.gpsimd.dma_start(
                        s1d[mm, :, q * QS:(q + 1) * QS], st[:]
                    )

            w2_tiles = {}

            def load_w2(m, eng):
                t = w2_pool.tile([128, MT, 128], F8, tag="w2", name=f"w2_{m}")
                for k0, k1 in ((0, 16), (16, MT)):
                    eng.dma_start(t[:, k0:k1, :], w2.ap()[m][:, k0:k1, :])
                w2_tiles[m] = t

            # ================= Phase 1: fc1 + prelu + stats ==================
            with contextlib.ExitStack() as es1:
                xt_pool = es1.enter_context(tc.tile_pool(name="xt", bufs=1))
                w1_pool = es1.enter_context(tc.tile_pool(name="w1p", bufs=2))
                p1_pool = es1.enter_context(tc.tile_pool(name="p1t", bufs=3))
                scr_pool = es1.enter_context(tc.tile_pool(name="scr1", bufs=2))
                # per-n tiles; first two n split finely so fc1 starts early
                xt_ts = []
                x8_ts = []
                for n in range(NB):
                    xt_n = xt_pool.tile([128, KC1, 512], F16, tag=f"xt{n}",
                                        name=f"xt{n}")
                    if n < 2:
                        splits = [(k, k + 1) for k in range(KC1)]
                    else:
                        splits = [(0, 5), (5, 9), (9, KC1)]
                    for k0, k1 in splits:
                        nc.sync.dma_start(
                            xt_n[:, k0:k1, :], xT.ap()[:, n, k0:k1, :]
                        )
                    xt_ts.append(xt_n)
                    x8_n = xt_pool.tile([128, 2 * NCORR, 512], F8,
                                        tag=f"x8{n}", name=f"x8{n}")
                    nc.sync.dma_start(x8_n[:], x8T.ap()[:, n, :, :])
                    x8_ts.append(x8_n)
                for m in range(MT):
                    emit_signs(8)
                    w1_t = w1_pool.tile([128, KC1, 128], F16, tag="w1")
                    w1c_t = w1_pool.tile([128, 2 * NCORR, 128], F8, tag="w1c")
                    if m < 2:
                        eng = nc.gpsimd
                        splits = [(k, k + 1) for k in range(KC1)]
                    else:
                        eng = nc.sync
                        splits = [(0, 7), (7, KC1)]
                    for k0, k1 in splits:
                        eng.dma_start(
                            w1_t[:, k0:k1, :], w1.ap()[m][:, k0:k1, :]
                        )
                    eng.dma_start(w1c_t[:], w1c.ap()[m])
                    for n in range(NB):
                        ps = ps_mm.tile([128, 512], F32, tag="mm")
                        for k in range(KC1):
                            nc.tensor.matmul(
                                ps[:], w1_t[:, k, :], xt_ts[n][:, k, :],
                                start=(k == 0), stop=False,
                            )
                        for j in range(NCORR):
                            nc.tensor.matmul(
                                ps[:], w1c_t[:, 2 * j:2 * j + 2, :],
                                x8_ts[n][:, 2 * j:2 * j + 2, :],
                                start=False, stop=(j == NCORR - 1),
                                perf_mode=mybir.MatmulPerfMode.DoubleRow,
                            )
                        p1_t = p1_pool.tile([128, 512], F32, tag="p1")
                        nc.scalar.activation(
                            p1_t[:], ps[:], AF.Prelu, alpha=a1_t[:],
                            scale=1.0 / FSPLIT,
                            accum_out=sums1[:, m, n:n + 1],
                        )
                        # p^2 sum on ScalarE (not DVE) so the vector FIFO stays
                        # free for BN finalize ops that wait on collectives
                        scr = scr_pool.tile([128, 512], F16, tag="scr")
                        nc.scalar.activation(
                            scr[:], p1_t[:], AF.Square,
                            accum_out=sq1[:, m, n:n + 1],
                        )
                        nc.sync.dma_start(
                            p1d[m, :, n * 512:(n + 1) * 512], p1_t[:]
                        )
                    for g in range(NGRP):
                        if m == GROUPS[g][1] - 1:
                            bn_reduce(sums1, sq1, cc_in1, cc_out1, g, "1")
                        if m == GROUPS[g][1] and g < NGRP - 1:
                            bn_finalize(g1_t, bt1_t, scale1, bias1, g, "1",
                                        negb1)
                            sign_group(g)
                    if m == 28:
                        load_w2(0, nc.sync)
                    if m == 29:
                        load_w2(1, nc.sync)
                emit_signs(8)
                # k >= KDIR tasks are handled in phase 2 (direct SBUF write)
                bn_finalize(g1_t, bt1_t, scale1, bias1, NGRP - 1, "1", negb1)

            # ============ Phase 2: fc2 + prelu + stats + fused fc3 ===========
            # m2-outer so W2 streams exactly once; s1 (fp8, 16.8 MB) is SBUF
            # resident (loads start the instant phase 1's xt pool frees).
            # fc3 partial chains interleave into the fc2 matmul stream.
            with contextlib.ExitStack() as es2:
                s1_pool = es2.enter_context(tc.tile_pool(name="s1", bufs=1))
                p2_pool = es2.enter_context(tc.tile_pool(name="p2t", bufs=4))
                scr2_pool = es2.enter_context(tc.tile_pool(name="sc2", bufs=3))
                q_pool = es2.enter_context(tc.tile_pool(name="qp", bufs=16))
                acc_pool = es2.enter_context(tc.tile_pool(name="acc", bufs=1))
                w3s_pool = es2.enter_context(tc.tile_pool(name="w3sp", bufs=1))
                ps3_pool = es2.enter_context(
                    tc.tile_pool(name="ps3", bufs=2, space="PSUM"))
                pcp_pool = es2.enter_context(
                    tc.tile_pool(name="pcp", bufs=1, space="PSUM"))
                pst_pool = es2.enter_context(
                    tc.tile_pool(name="pst", bufs=1, space="PSUM"))
                sm_pool = es2.enter_context(tc.tile_pool(name="sm", bufs=1))
                out_pool = es2.enter_context(tc.tile_pool(name="op", bufs=1))
                s1_t = s1_pool.tile([128, MT, BS], F8, tag="s1")
                for k in range(KDIR):
                    for h in range(2):
                        nc.sync.dma_start(
                            s1_t[:, k, h * 2048:(h + 1) * 2048],
                            s1d[k, :, h * 2048:(h + 1) * 2048],
                        )
                # k=28,29: +-1 via ScalarE Sign (stats ready; never waits)
                for mm in (28, 29):
                    for q in range(BS // QS):
                        pin = pin_pool.tile([128, QS], F32, tag="pin",
                                            name=f"pind_{mm}_{q}")
                        nc.gpsimd.dma_start(
                            pin[:], p1d[mm, :, q * QS:(q + 1) * QS]
                        )
                        nc.scalar.activation(
                            s1_t[:, mm, q * QS:(q + 1) * QS], pin[:], AF.Sign,
                            bias=bias1[:, mm:mm + 1],
                            scale=scale1[:, mm:mm + 1],
                        )
                # k=30,31: {0,1} via DVE is_ge (waits the last BN1 AllReduce
                # on the otherwise-idle vector FIFO)
                for mm in (30, 31):
                    for q in range(BS // QS):
                        pin = pin_pool.tile([128, QS], F32, tag="pin",
                                            name=f"pind_{mm}_{q}")
                        nc.gpsimd.dma_start(
                            pin[:], p1d[mm, :, q * QS:(q + 1) * QS]
                        )
                        nc.vector.tensor_scalar(
                            s1_t[:, mm, q * QS:(q + 1) * QS], pin[:],
                            scale1[:, mm:mm + 1], negb1[:, mm:mm + 1],
                            ALU.mult, ALU.is_ge,
                        )
                acc_t = acc_pool.tile([C, NB, 512], F32, tag="acc")
                w3s_t = w3s_pool.tile([128, MT, C], F16, tag="w3s")

                def bn2_extras(g):
                    """Per-group BN2-fold: w3s = scale2*W3, corr += W3^T bias2."""
                    m0, m1 = GROUPS[g]
                    nc.vector.tensor_copy(bias2h[:, m0:m1], bias2[:, m0:m1])
                    for k in range(m0, m1):
                        nc.vector.tensor_scalar_mul(
                            w3s_t[:, k, :], w3_t[:, k, :], scale2[:, k:k + 1]
                        )
                    pcp = pcp_pool.tile([C, 1], F32, tag="pcp")
                    for i, k in enumerate(range(m0, m1)):
                        nc.tensor.matmul(
                            pcp[:], w3_t[:, k, :], bias2h[:, k:k + 1],
                            start=(i == 0), stop=(k == m1 - 1),
                        )
                    if g == 0:
                        nc.vector.tensor_copy(corr_t[:], pcp[:])
                    else:
                        nc.vector.tensor_add(corr_t[:], corr_t[:], pcp[:])

                fc3_pend = []

                def emit_fc3_chain(g, n):
                    m0, m1 = GROUPS[g]
                    qts = []
                    for k in range(m0, m1):
                        qt = q_pool.tile([128, 512], F16, tag="q",
                                         name=f"q_{g}_{n}_{k}")
                        nc.gpsimd.dma_start(
                            qt[:], p2d[k, :, n * 512:(n + 1) * 512]
                        )
                        qts.append(qt)
                    pl = ps3_pool.tile([C, 512], F32, tag="pl")
                    for i, k in enumerate(range(m0, m1)):
                        nc.tensor.matmul(
                            pl[:], w3s_t[:, k, :], qts[i][:],
                            start=(i == 0), stop=(k == m1 - 1),
                        )
                    if g == 0:
                        nc.vector.tensor_copy(acc_t[:, n, :], pl[:])
                    else:
                        nc.vector.tensor_add(acc_t[:, n, :], acc_t[:, n, :],
                                             pl[:])

                for m in range(MT):
                    if m not in w2_tiles:
                        load_w2(m, nc.sync)
                    w2_t = w2_tiles.pop(m)
                    for n_g in range(NB):
                        ps = ps_mm.tile([128, 512], F32, tag="mm")
                        for kk in range(MT // 2):
                            nc.tensor.matmul(
                                ps[:], w2_t[:, 2 * kk:2 * kk + 2, :],
                                s1_t[:, 2 * kk:2 * kk + 2,
                                     n_g * 512:(n_g + 1) * 512],
                                start=(kk == 0), stop=(kk == MT // 2 - 1),
                                perf_mode=mybir.MatmulPerfMode.DoubleRow,
                            )
                        p2_t = p2_pool.tile([128, 512], F16, tag="p2")
                        # h2 = 2*(u @ sW2^T) - colsum(sW2): exact (even ints)
                        nc.scalar.activation(
                            p2_t[:], ps[:], AF.Prelu, alpha=a2_t[:],
                            scale=2.0, bias=c2n_t[:, m:m + 1],
                            accum_out=sums2[:, m, n_g:n_g + 1],
                        )
                        scr = scr2_pool.tile([128, 512], F16, tag="scr2")
                        nc.scalar.activation(
                            scr[:], p2_t[:], AF.Square,
                            accum_out=sq2[:, m, n_g:n_g + 1],
                        )
                        nc.sync.dma_start(
                            p2d[m, :, n_g * 512:(n_g + 1) * 512], p2_t[:]
                        )
                    for g in range(NGRP):
                        if m == GROUPS[g][1] - 1:
                            bn_reduce(sums2, sq2, cc_in2, cc_out2, g, "2")
                        if m == GROUPS[g][1] and g < NGRP - 1:
                            bn_finalize(g2_t, bt2_t, scale2, bias2, g, "2")
                            bn2_extras(g)
                        if m == GROUPS[g][1] + 1 and g < NGRP - 1:
                            fc3_pend.extend((g, n) for n in range(NB))
                    cap = len(fc3_pend) if m == MT - 1 else 3
                    for _ in range(min(cap, len(fc3_pend))):
                        emit_fc3_chain(*fc3_pend.pop(0))

                # ---------- tail: last-group fc3 + log_softmax ----------------
                bn_finalize(g2_t, bt2_t, scale2, bias2, NGRP - 1, "2")
                bn2_extras(NGRP - 1)
                fc3_pend.extend((NGRP - 1, n) for n in range(NB))
                for g, n in fc3_pend:
                    emit_fc3_chain(g, n)
                fc3_pend = []
                # ---- bulk log_softmax on one PE-transposed [128, 32, C]
                # block.  Logits are O(+-8) so exp() needs no max-shift in
                # fp32; the per-row lse subtraction rides the ACT bias port.
                nc.vector.tensor_add(b3c_t[:], b3_t[:], corr_t[:])
                nc.vector.tensor_scalar(
                    acc_t[:], acc_t[:], b3c_t[:], None, ALU.add
                )
                JJ = 4 * NB
                ptall = pst_pool.tile([128, JJ, C], F32, tag="pt")
                for n in range(NB):
                    for j in range(4):
                        nc.tensor.transpose(
                            ptall[:, n * 4 + j, :],
                            acc_t[:, n, j * 128:(j + 1) * 128], eye_t[:]
                        )
                ex2 = sm_pool.tile([128, JJ, C], F32, tag="ex2")
                nc.scalar.activation(ex2[:], ptall[:], AF.Exp)
                sen = sm_pool.tile([128, JJ], F32, tag="se")
                nc.vector.reduce_sum(sen[:], ex2[:], axis=mybir.AxisListType.X)
                lnn = sm_pool.tile([128, JJ], F32, tag="ln")
                nc.scalar.activation(lnn[:], sen[:], AF.Ln)
                nln = sm_pool.tile([128, JJ], F32, tag="nln")
                nc.vector.tensor_scalar_mul(nln[:], lnn[:], -1.0)
                ot = out_pool.tile([128, JJ, C], F32, tag="ot")
                for jj in range(JJ):
                    nc.scalar.activation(
                        ot[:, jj, :], ptall[:, jj, :], AF.Identity,
                        bias=nln[:, jj:jj + 1],
                    )
                nc.sync.dma_start(out.ap(), ot[:])

    nc.compile()
    return nc


def prep_inputs(x, W1, b1, a1, g1, beta1, W2, a2, g2, beta2, W3, b3):
    """Host-side layout prep. Returns per-core in_maps."""
    x = np.ascontiguousarray(np.asarray(x, np.float32))
    W1 = np.asarray(W1, np.float32)
    b1 = np.asarray(b1, np.float32)
    W2 = np.asarray(W2, np.float32)
    W3 = np.asarray(W3, np.float32)
    b3 = np.asarray(b3, np.float32)

    # fc1 operands with bias folded in as contraction row 784 (rows 785+ zero).
    # fp16 2-term hi/lo split with 2^18 scaling, packed along K:
    #   XF = [xh; xl*S],  WF = [wh*S; wh]  ->  psum ~= S * (x @ w1f)
    # with w1f = fp16(w*S)/S.  The dropped x*(w - w1f) ~ 2^-12 term is
    # recovered by an fp8 DoubleRow correction xh8 @ (wl*S) over rows 0:768,
    # shrinking h1 error to ~2e-5 rms.  S = 2^18 keeps wl*S and xh8 inside
    # fp8e4 normal range while w*S stays under fp16 max (|w| < 0.25).
    # The bias row uses x-side 1024.0 / w-side b1/1024.
    S = np.float32(FSPLIT)
    xT_aug = np.zeros((D + 1, B), np.float32)
    xT_aug[0:D] = x.T
    xT_aug[D] = 1024.0
    w1T_aug = np.zeros((D + 1, H1), np.float32)
    w1T_aug[0:D] = W1.T
    w1T_aug[D] = b1 / 1024.0

    xh = xT_aug.astype(np.float16)
    xls = ((xT_aug - xh.astype(np.float32)) * S).astype(np.float16)
    whs = np.clip(w1T_aug * S, -65504.0, 65504.0).astype(np.float16)
    w1f = whs.astype(np.float32) / S          # exact value term 1 multiplies
    wh = w1f.astype(np.float16)
    CPAD = 2 * NCORR * 128
    wl_s = np.zeros((CPAD, H1), ml_dtypes.float8_e4m3)
    wl_s[0:D + 1] = ((w1T_aug - w1f) * S).astype(ml_dtypes.float8_e4m3)
    # bias row D excluded: fp8(1024) overflows e4m3; its residual is
    # b1's fp16 rounding only (b1/1024 is exact for typical b1).
    x8 = np.zeros((CPAD, B), ml_dtypes.float8_e4m3)
    x8[0:D] = xT_aug[0:D].astype(ml_dtypes.float8_e4m3)
    KPAD = KC1 * 128
    A = D + 1
    xF = np.zeros((KPAD, B), np.float16)
    xF[0:A] = xh
    xF[A:2 * A] = xls
    wF = np.zeros((KPAD, H1), np.float16)
    wF[0:A] = whs
    wF[A:2 * A] = wh
    w1_blk = np.ascontiguousarray(
        wF.reshape(KC1, 128, MT, 128).transpose(2, 1, 0, 3)
    )
    w1c_blk = np.ascontiguousarray(
        wl_s.reshape(2 * NCORR, 128, MT, 128).transpose(2, 1, 0, 3)
    )

    # k < 28 and k in {30,31}: s1 encoded {0,1}, weights +-1, corrected via
    # c2n = -colsum.  k in {28,29}: s1 encoded +-1 with weights halved to
    # +-0.5 (the fc2 epilogue applies a uniform scale of 2).
    sW2T = np.where(W2 >= 0, np.float32(1), np.float32(-1)).T
    sW2Ts = sW2T.copy()
    sW2Ts[28 * 128:30 * 128] *= np.float32(0.5)
    w2_blk = np.ascontiguousarray(
        sW2Ts.reshape(MT, 128, MT, 128).transpose(2, 1, 0, 3)
    ).astype(ml_dtypes.float8_e4m3)
    c2n_blk = -(
        sW2T[0:28 * 128].sum(axis=0, dtype=np.float64)
        + sW2T[30 * 128:].sum(axis=0, dtype=np.float64)
    ).astype(np.float32)

    w3_blk = np.ascontiguousarray(
        W3.T.reshape(MT, 128, C).transpose(1, 0, 2)
    ).astype(np.float16)

    def feat_layout(v):
        return np.ascontiguousarray(np.asarray(v, np.float32).reshape(MT, 128).T)

    shared = dict(
        w1=w1_blk, w1c=w1c_blk, w2=w2_blk, w3=w3_blk,
        g1=feat_layout(g1), bt1=feat_layout(beta1),
        g2=feat_layout(g2), bt2=feat_layout(beta2),
        a1p=np.full((128, 1), np.float32(a1), np.float32),
        a2p=np.full((128, 1), np.float32(a2), np.float32),
        b3p=b3.reshape(C, 1).astype(np.float32),
        c2n=feat_layout(c2n_blk),
        eye=np.eye(C, dtype=np.float32),
    )
    in_maps = []
    for c in range(NCORES):
        sl = xF[:, c * BS:(c + 1) * BS]
        xs = np.ascontiguousarray(
            sl.reshape(KC1, 128, NB, 512).transpose(1, 2, 0, 3)
        )
        x8s = np.ascontiguousarray(
            x8[:, c * BS:(c + 1) * BS]
            .reshape(2 * NCORR, 128, NB, 512).transpose(1, 2, 0, 3)
        )
        in_maps.append(dict(shared, xT=xs, x8T=x8s))
    return in_maps


_NC_CACHE = {}


def run(inputs, debug=False, trace=False):
    key = (debug,)
    if key not in _NC_CACHE:
        _NC_CACHE[key] = build_program(debug=debug)
    nc = _NC_CACHE[key]
    in_maps = prep_inputs(**inputs)
    res = run_bass_kernel_spmd(
        nc, in_maps, core_ids=list(range(NCORES)), trace=trace
    )
    # out is [128, 32, C] partition-major; row jj*128+p <-> out[p, jj]
    outs = np.concatenate([
        np.transpose(res.results[c]["out"], (1, 0, 2)).reshape(BS, C)
        for c in range(NCORES)
    ], axis=0)
    return outs, res


def kernel(**inputs):
    out, _ = run(inputs)
    return out

